# revision 5
# baseline (speedup 1.0000x reference)
import sys

sys.path.insert(0, "/opt/trn_rl_repo")

import ml_dtypes
import numpy as np

N_CORES = 8
B, T, C = 2, 2048, 1024
H, D = 16, 64
HPC = H // N_CORES          # heads per core = 2
CPC = HPC * D               # channels per core = 128
NK = C // 128               # k-tiles = 8

# consts layout (bf16 tensor)
C_ID = 0                    # identity [128,128] (PE transpose)
C_T2 = 128                  # [tri01 | tri01]  [128,256]
C_C1 = 384                  # value 1.0 block [128,64] (bc stationary + v ones)
CW_BF = 448

_CACHE = {}
LAST_EXEC_NS = None


def _build():
    import concourse.tile as tile
    from concourse import bacc, mybir

    f32 = mybir.dt.float32
    bf16 = mybir.dt.bfloat16
    Exp = mybir.ActivationFunctionType.Exp
    MUL = mybir.AluOpType.mult

    nc = bacc.Bacc(None, num_devices=N_CORES)

    xT_in = nc.declare_dram_parameter("xT", [128, NK, B * T], bf16, isOutput=False)
    wq_in = nc.declare_dram_parameter("wq", [128, NK, CPC], bf16, isOutput=False)
    wk_in = nc.declare_dram_parameter("wk", [128, NK, CPC], bf16, isOutput=False)
    wv_in = nc.declare_dram_parameter("wv", [128, NK, CPC], bf16, isOutput=False)
    wp_in = nc.declare_dram_parameter("wp", [C, C], bf16, isOutput=False)
    cs_in = nc.declare_dram_parameter("consts", [128, CW_BF], bf16, isOutput=False)
    y_out = nc.declare_dram_parameter("y", [4 * 128, C], bf16, isOutput=True)

    with tile.TileContext(nc) as tc:
        with tc.tile_pool(name="ps", bufs=1, space="PSUM") as ps, \
             tc.tile_pool(name="dram", bufs=1, space="DRAM") as dram, \
             tc.tile_pool(name="sb", bufs=1) as sb:

            # ---- persistent SBUF tiles ----
            qT = sb.tile([128, B * T], bf16, name="qT")
            kT = sb.tile([128, B * T], bf16, name="kT")
            VB = 80
            v_nat = sb.tile([128, B * 16, 2 * VB], bf16, name="v_nat")
            cons = sb.tile([128, CW_BF], bf16, name="cons")
            ident = cons[:, C_ID:C_ID + 128]
            tri2 = cons[:, C_T2:C_T2 + 256]
            c1w = cons[0:1, C_C1:C_C1 + 64]

            wq_sb = sb.tile([128, NK, CPC], bf16, name="wq_sb")
            wk_sb = sb.tile([128, NK, CPC], bf16, name="wk_sb")
            wv_sb = sb.tile([128, NK, CPC], bf16, name="wv_sb")
            wp_sb = sb.tile([128, NK, C], bf16, name="wp_sb")
            a2a_sb = [sb.tile([128, NK, 128], bf16, name=f"a2a_sb{e}")
                      for e in range(4)]

            send_d = [dram.tile([N_CORES, 128, 128], bf16, name=f"send_d{e}")
                      for e in range(4)]
            recv_d = [dram.tile([N_CORES, 128, 128], bf16, name=f"recv_d{e}")
                      for e in range(4)]

            # warm the exp table load at t=0 so it never gates real exps
            warm = sb.tile([1, 8], f32, name="warm")
            warm2 = sb.tile([1, 8], f32, name="warm2")
            nc.vector.memset(warm, 0.0)
            nc.scalar.activation(out=warm2, in_=warm, func=Exp, scale=1.0)

            # ---- initial DMAs (keep the scalar queue free for ACT) ----
            nc.sync.dma_start(out=cons, in_=cs_in[:])
            nc.sync.dma_start(out=wq_sb, in_=wq_in[:])
            nc.sync.dma_start(out=wk_sb, in_=wk_in[:])
            nc.gpsimd.dma_start(out=wv_sb, in_=wv_in[:])
            nc.gpsimd.dma_start(out=v_nat[:, :, D:D + 1],
                                in_=cs_in[:, C_C1:C_C1 + B * 16])
            nc.gpsimd.dma_start(out=v_nat[:, :, VB + D:VB + D + 1],
                                in_=cs_in[:, C_C1:C_C1 + B * 16])

            xt0 = sb.tile([128, NK, 512], bf16, name="xt0")
            nc.sync.dma_start(out=xt0[:, 0:4, :], in_=xT_in[:, 0:4, 0:512])
            nc.gpsimd.dma_start(out=xt0[:, 4:8, :], in_=xT_in[:, 4:8, 0:512])
            for k in range(NK):
                nc.gpsimd.dma_start(out=wp_sb[:, k, :],
                                    in_=wp_in[128 * k:128 * (k + 1), :])

            # ---------------- qkv chunk: PE-op thunks ----------------
            def qkv_ops(b, tch, xt):
                col = b * T + 512 * tch
                ops = []

                def series(w_sb, dstT, is_v):
                    acc = [None]

                    def mm(k):
                        def run():
                            if k == 0:
                                acc[0] = ps.tile([128, 512], f32, tag="acc",
                                                 bufs=2, name="qacc")
                            nc.tensor.matmul(acc[0],
                                             w_sb[:, k, :], xt[:, k, :],
                                             start=(k == 0), stop=(k == NK - 1))
                            if k == NK - 1:
                                with nc.allow_low_precision(reason="bf16 ok"):
                                    if not is_v:
                                        nc.vector.tensor_copy(
                                            out=dstT[:, col:col + 512], in_=acc[0])
                                    else:
                                        vtmp = sb.tile([128, 512], bf16,
                                                       tag="vtmp", bufs=3)
                                        nc.vector.tensor_copy(out=vtmp, in_=acc[0])
                                        for kk in range(4):
                                            kb = 4 * tch + kk
                                            tr = ps.tile([128, 128], bf16,
                                                         tag="acc", bufs=2,
                                                         name="tr")
                                            nc.tensor.transpose(
                                                tr, vtmp[:, 128 * kk:128 * (kk + 1)],
                                                ident)
                                            for hl in range(HPC):
                                                nc.vector.tensor_copy(
                                                    out=v_nat[:, 16 * b + kb,
                                                              VB * hl:VB * hl + D],
                                                    in_=tr[:, D * hl:D * (hl + 1)])
                        return run
                    return [mm(k) for k in range(NK)]

                ops += series(wq_sb, qT, False)
                ops += series(wk_sb, kT, False)
                ops += series(wv_sb, None, True)
                return ops

            # ---------------- proj for one exchange (b, jp) ----------------
            def proj_ops(b, jp):
                e = 2 * b + jp
                ops = []
                for cc in range(2):
                    yp = [None]

                    def mm(k, cc=cc, yp=yp):
                        def run():
                            if k == 0:
                                yp[0] = ps.tile([128, 512], f32, tag="acc",
                                                bufs=2, name="yp")
                            nc.tensor.matmul(yp[0],
                                             a2a_sb[e][:, k, :],
                                             wp_sb[:, k, 512 * cc:512 * (cc + 1)],
                                             start=(k == 0), stop=(k == NK - 1))
                            if k == NK - 1:
                                ysb = sb.tile([128, 512], bf16, tag="ysb", bufs=4)
                                with nc.allow_low_precision(reason="bf16 out"):
                                    nc.vector.tensor_copy(out=ysb, in_=yp[0])
                                nc.sync.dma_start(
                                    out=y_out[128 * e:128 * (e + 1),
                                              512 * cc:512 * (cc + 1)],
                                    in_=ysb)
                        return run
                    ops += [mm(k) for k in range(NK)]
                return ops

            pend = []
            proj_fill = []

            def a2a(e):
                nc.gpsimd.collective_compute(
                    "AllToAll", mybir.AluOpType.bypass,
                    replica_groups=[list(range(N_CORES))],
                    ins=[send_d[e].opt()], outs=[recv_d[e].opt()])

            def do_apply(w):
                avp, sums, b, j = w
                bc = ps.tile([128, 512], f32, tag="acc", bufs=2, name="bc")
                for h in range(HPC):
                    rec = sb.tile([1, 512], f32, tag="rec", bufs=4, name="rec")
                    rec_bf = sb.tile([1, 512], bf16, tag="recb", bufs=4, name="recb")
                    with nc.allow_low_precision(reason="softmax denom"):
                        nc.vector.reciprocal_approx_fast(out=rec, in_=sums[h])
                        nc.vector.tensor_copy(out=rec_bf, in_=rec)
                    nc.tensor.matmul(bc[64 * h:64 * (h + 1), :], c1w, rec_bf,
                                     start=True, stop=True)
                attn_w = sb.tile([128, 4, 128], bf16, tag="attnw", bufs=4,
                                 name="attnw")
                with nc.allow_low_precision(reason="bf16 attn"):
                    nc.vector.tensor_tensor(out=attn_w, in0=avp, in1=bc, op=MUL)
                jp = j // 2
                e = 2 * b + jp
                nc.gpsimd.dma_start(
                    out=send_d[e][4 * (j % 2):4 * (j % 2) + 4].transpose([1, 0, 2]),
                    in_=attn_w)
                if j % 2 == 1:
                    a2a(e)
                    nc.gpsimd.dma_start(out=a2a_sb[e],
                                        in_=recv_d[e][:].transpose([1, 0, 2]))
                    proj_fill.extend(proj_ops(b, jp))

            # ---------------- pipelined schedule ----------------
            steps = [(b, t) for b in range(B) for t in range(4)]

            for idx in range(9):
                fill = list(proj_fill)
                proj_fill.clear()
                if idx < 8:
                    b, tch = steps[idx]
                    if idx == 0:
                        xt = xt0
                    else:
                        col = b * T + 512 * tch
                        xt = sb.tile([128, NK, 512], bf16, tag="xt", bufs=3)
                        nc.sync.dma_start(out=xt, in_=xT_in[:, :, col:col + 512])
                    fill = qkv_ops(b, tch, xt) + fill
                fi = [0]

                def pull(n, fill=fill, fi=fi):
                    for _ in range(n):
                        if fi[0] < len(fill):
                            fill[fi[0]]()
                            fi[0] += 1

                if idx == 0:
                    pull(len(fill))
                else:
                    wb, wj = steps[idx - 1]
                    qcol = wb * T + 512 * wj
                    npairs = 2 * (wj + 1)
                    per = max(1, len(fill) // (2 * npairs + 1))
                    av_ps = None
                    for s in range(npairs):
                        P = sb.tile([128, 2, 2, 512], bf16, tag="p", bufs=6,
                                    name="P")
                        for i2 in range(2):
                            m = 2 * s + i2
                            diag = m >= 4 * wj
                            lo = 128 * (m - 4 * wj) if diag else 0
                            sp = ps.tile([128, 2, 512], f32, tag="sp", bufs=2,
                                         name="sp")
                            kcol = wb * T + 128 * m
                            for h in range(HPC):
                                nc.tensor.matmul(
                                    sp[:, h, lo:512],
                                    kT[64 * h:64 * h + D, kcol:kcol + 128],
                                    qT[64 * h:64 * h + D, qcol + lo:qcol + 512],
                                    start=True, stop=True)
                            nc.scalar.activation(out=P[:, i2, :, lo:512],
                                                 in_=sp[:, :, lo:512],
                                                 func=Exp, scale=0.125)
                            if diag:
                                with nc.allow_low_precision(reason="0/1 mask"):
                                    nc.vector.tensor_tensor(
                                        out=P[:, i2, :, lo:lo + 128],
                                        in0=P[:, i2, :, lo:lo + 128],
                                        in1=tri2, op=MUL)
                            pull(per)
                            if s == 0 and i2 == 0:
                                av_ps = [ps.tile([128, 512], f32, tag="av",
                                                 bufs=2, name=f"av{h}")
                                         for h in range(HPC)]
                            for h in range(HPC):
                                nc.tensor.matmul(
                                    av_ps[h][0:D + 1, lo:512],
                                    v_nat[:, 16 * wb + m, VB * h:VB * h + D + 1],
                                    P[:, i2, h, lo:512],
                                    start=(m == 0), stop=(m == 2 * npairs - 1))
                        if s == 0 and pend:
                            do_apply(pend.pop(0))
                    pull(len(fill))
                    avp = sb.tile([128, 512], bf16, tag="avp", bufs=2, name="avp")
                    sums = [sb.tile([1, 512], f32, tag="sums", bufs=4, name="sums")
                            for _ in range(HPC)]
                    with nc.allow_low_precision(reason="bf16 ok"):
                        for h in range(HPC):
                            nc.vector.tensor_copy(out=avp[64 * h:64 * h + D, :],
                                                  in_=av_ps[h][0:D, :])
                            nc.vector.tensor_copy(out=sums[h],
                                                  in_=av_ps[h][D:D + 1, :])
                    pend.append((avp, sums, wb, wj))

            # tail: apply last window, final exchange, remaining proj
            fillt = list(proj_fill)
            proj_fill.clear()
            half = len(fillt) // 2
            for op in fillt[:half]:
                op()
            do_apply(pend.pop(0))           # (1,3) -> fires a2a(1,1)
            for op in fillt[half:]:
                op()
            for op in proj_fill:            # proj(1,1)
                op()

    nc.finalize()
    return nc


def kernel(x, Wq, Wk, Wv, Wproj, bproj):
    global LAST_EXEC_NS
    from concourse.bass_utils import run_bass_kernel_spmd

    if "nc" not in _CACHE:
        _CACHE["nc"] = _build()
    nc = _CACHE["nc"]

    bf = ml_dtypes.bfloat16
    xT = np.ascontiguousarray(
        x.reshape(B * T, NK, 128).transpose(2, 1, 0)).astype(bf)
    wp = np.ascontiguousarray(Wproj).astype(bf)

    consts = np.zeros((128, CW_BF), dtype=np.float32)
    consts[:, C_ID:C_ID + 128] = np.eye(128)
    pi = np.arange(128)[:, None]
    ci = np.arange(128)[None, :]
    tri01 = (ci - pi >= 0).astype(np.float32)
    consts[:, C_T2:C_T2 + 128] = tri01
    consts[:, C_T2 + 128:C_T2 + 256] = tri01
    consts[:, C_C1:C_C1 + 64] = 1.0
    consts = consts.astype(bf)

    def rearrange_w(w):
        # [C, CPC] -> [128, NK, CPC]
        return np.ascontiguousarray(
            w.reshape(NK, 128, CPC).transpose(1, 0, 2)).astype(bf)

    in_maps = []
    for c in range(N_CORES):
        in_maps.append({
            "xT": xT,
            "wq": rearrange_w(np.concatenate([Wq[2 * c], Wq[2 * c + 1]], axis=1)),
            "wk": rearrange_w(np.concatenate([Wk[2 * c], Wk[2 * c + 1]], axis=1)),
            "wv": rearrange_w(np.concatenate([Wv[2 * c], Wv[2 * c + 1]], axis=1)),
            "wp": wp,
            "consts": consts,
        })

    res = run_bass_kernel_spmd(nc, in_maps, list(range(N_CORES)))
    LAST_EXEC_NS = res.exec_time_ns
    y = np.empty((B, T, C), dtype=np.float32)
    for c in range(N_CORES):
        yc = np.asarray(res.results[c]["y"]).astype(np.float32) + bproj[None, :]
        for b in range(B):
            for jp in range(2):
                y[b, 1024 * jp + 128 * c:1024 * jp + 128 * c + 128, :] = \
                    yc[128 * (2 * b + jp):128 * (2 * b + jp + 1), :]
    return y


# revision 11
# speedup vs baseline: 1.0142x; 1.0142x over previous
import sys

sys.path.insert(0, "/opt/trn_rl_repo")

import ml_dtypes
import numpy as np

N_CORES = 8
B, T, C = 2, 2048, 1024
H, D = 16, 64
HPC = H // N_CORES          # heads per core = 2
CPC = HPC * D               # channels per core = 128
NK = C // 128               # k-tiles = 8

# consts layout (bf16 tensor)
C_ID = 0                    # identity [128,128] (PE transpose)
C_T2 = 128                  # [tri01 | tri01]  [128,256]
C_C1 = 384                  # value 1.0 block [128,64] (bc stationary + v ones)
CW_BF = 448

_CACHE = {}
LAST_EXEC_NS = None


def _build():
    import concourse.tile as tile
    from concourse import bacc, mybir

    f32 = mybir.dt.float32
    bf16 = mybir.dt.bfloat16
    Exp = mybir.ActivationFunctionType.Exp
    MUL = mybir.AluOpType.mult

    nc = bacc.Bacc(None, num_devices=N_CORES)

    xT_in = nc.declare_dram_parameter("xT", [128, NK, B * T], bf16, isOutput=False)
    wq_in = nc.declare_dram_parameter("wq", [128, NK, CPC], bf16, isOutput=False)
    wk_in = nc.declare_dram_parameter("wk", [128, NK, CPC], bf16, isOutput=False)
    wv_in = nc.declare_dram_parameter("wv", [128, NK, CPC], bf16, isOutput=False)
    wp_in = nc.declare_dram_parameter("wp", [C, C], bf16, isOutput=False)
    cs_in = nc.declare_dram_parameter("consts", [128, CW_BF], bf16, isOutput=False)
    y_out = nc.declare_dram_parameter("y", [4 * 128, C], bf16, isOutput=True)

    with tile.TileContext(nc) as tc:
        with tc.tile_pool(name="ps", bufs=1, space="PSUM") as ps, \
             tc.tile_pool(name="dram", bufs=1, space="DRAM") as dram, \
             tc.tile_pool(name="sb", bufs=1) as sb:

            # ---- persistent SBUF tiles ----
            qT = sb.tile([128, B * T], bf16, name="qT")
            kT = sb.tile([128, B * T], bf16, name="kT")
            VB = 80
            v_nat = sb.tile([128, B * 16, 2 * VB], bf16, name="v_nat")
            cons = sb.tile([128, CW_BF], bf16, name="cons")
            ident = cons[:, C_ID:C_ID + 128]
            tri2 = cons[:, C_T2:C_T2 + 256]
            c1w = cons[0:1, C_C1:C_C1 + 64]

            wq_sb = sb.tile([128, NK, CPC], bf16, name="wq_sb")
            wk_sb = sb.tile([128, NK, CPC], bf16, name="wk_sb")
            wv_sb = sb.tile([128, NK, CPC], bf16, name="wv_sb")
            wp_sb = sb.tile([128, NK, C], bf16, name="wp_sb")
            a2a_sb = [sb.tile([128, NK, 128], bf16, name=f"a2a_sb{e}")
                      for e in range(4)]

            send_d = [dram.tile([N_CORES, 128, 128], bf16, name=f"send_d{e}")
                      for e in range(4)]
            recv_d = [dram.tile([N_CORES, 128, 128], bf16, name=f"recv_d{e}")
                      for e in range(4)]

            # warm the exp table load at t=0 so it never gates real exps
            warm = sb.tile([1, 8], f32, name="warm")
            warm2 = sb.tile([1, 8], f32, name="warm2")
            nc.vector.memset(warm, 0.0)
            nc.scalar.activation(out=warm2, in_=warm, func=Exp, scale=1.0)

            # ---- initial DMAs ----
            # gpsimd (software DGE) carries ONLY collective triggers: bulk
            # DMAs ride the two hardware DGE queues (sync + scalar).
            xt0 = sb.tile([128, NK, 512], bf16, name="xt0")
            nc.sync.dma_start(out=cons, in_=cs_in[:])
            nc.sync.dma_start(out=wq_sb, in_=wq_in[:])
            for k in range(4):
                nc.sync.dma_start(out=xt0[:, k, :], in_=xT_in[:, k, 0:512])
                nc.scalar.dma_start(out=xt0[:, k + 4, :],
                                    in_=xT_in[:, k + 4, 0:512])
            nc.sync.dma_start(out=wk_sb, in_=wk_in[:])
            nc.sync.dma_start(out=wv_sb, in_=wv_in[:])
            for k in range(NK):
                nc.scalar.dma_start(out=wp_sb[:, k, :],
                                    in_=wp_in[128 * k:128 * (k + 1), :])
            # v ones columns straight from the consts block via DVE
            with nc.allow_low_precision(reason="const copy"):
                nc.vector.tensor_copy(out=v_nat[:, :, D:D + 1],
                                      in_=cons[:, C_C1:C_C1 + B * 16])
                nc.vector.tensor_copy(out=v_nat[:, :, VB + D:VB + D + 1],
                                      in_=cons[:, C_C1:C_C1 + B * 16])

            # ---------------- qkv chunk: PE-op thunks ----------------
            def qkv_ops(b, tch, xt):
                col = b * T + 512 * tch
                ops = []

                def series(w_sb, dstT, is_v):
                    acc = [None]

                    def mm(k):
                        def run():
                            if k == 0:
                                acc[0] = ps.tile([128, 512], f32, tag="acc",
                                                 bufs=2, name="qacc")
                            nc.tensor.matmul(acc[0],
                                             w_sb[:, k, :], xt[:, k, :],
                                             start=(k == 0), stop=(k == NK - 1))
                            if k == NK - 1:
                                with nc.allow_low_precision(reason="bf16 ok"):
                                    if not is_v:
                                        nc.vector.tensor_copy(
                                            out=dstT[:, col:col + 512], in_=acc[0])
                                    else:
                                        vtmp = sb.tile([128, 512], bf16,
                                                       tag="vtmp", bufs=3)
                                        nc.vector.tensor_copy(out=vtmp, in_=acc[0])
                                        for kk in range(4):
                                            kb = 4 * tch + kk
                                            tr = ps.tile([128, 128], bf16,
                                                         tag="acc", bufs=2,
                                                         name="tr")
                                            nc.tensor.transpose(
                                                tr, vtmp[:, 128 * kk:128 * (kk + 1)],
                                                ident)
                                            for hl in range(HPC):
                                                nc.vector.tensor_copy(
                                                    out=v_nat[:, 16 * b + kb,
                                                              VB * hl:VB * hl + D],
                                                    in_=tr[:, D * hl:D * (hl + 1)])
                        return run
                    return [mm(k) for k in range(NK)]

                ops += series(wq_sb, qT, False)
                ops += series(wk_sb, kT, False)
                ops += series(wv_sb, None, True)
                return ops

            # ---------------- proj for one exchange (b, jp) ----------------
            def proj_ops(b, jp):
                e = 2 * b + jp
                ops = []
                for cc in range(2):
                    yp = [None]

                    def mm(k, cc=cc, yp=yp):
                        def run():
                            if k == 0:
                                yp[0] = ps.tile([128, 512], f32, tag="acc",
                                                bufs=2, name="yp")
                            nc.tensor.matmul(yp[0],
                                             a2a_sb[e][:, k, :],
                                             wp_sb[:, k, 512 * cc:512 * (cc + 1)],
                                             start=(k == 0), stop=(k == NK - 1))
                            if k == NK - 1:
                                ysb = sb.tile([128, 512], bf16, tag="ysb", bufs=4)
                                with nc.allow_low_precision(reason="bf16 out"):
                                    nc.vector.tensor_copy(out=ysb, in_=yp[0])
                                nc.sync.dma_start(
                                    out=y_out[128 * e:128 * (e + 1),
                                              512 * cc:512 * (cc + 1)],
                                    in_=ysb)
                        return run
                    ops += [mm(k) for k in range(NK)]
                return ops

            pend = []
            ready = {}

            def a2a(e):
                nc.gpsimd.collective_compute(
                    "AllToAll", mybir.AluOpType.bypass,
                    replica_groups=[list(range(N_CORES))],
                    ins=[send_d[e].opt()], outs=[recv_d[e].opt()])

            def do_apply(w, cur_idx):
                avp, sums, b, j = w
                bc = ps.tile([128, 512], f32, tag="acc", bufs=2, name="bc")
                for h in range(HPC):
                    rec = sb.tile([1, 512], f32, tag="rec", bufs=4, name="rec")
                    rec_bf = sb.tile([1, 512], bf16, tag="recb", bufs=4, name="recb")
                    with nc.allow_low_precision(reason="softmax denom"):
                        nc.vector.reciprocal_approx_fast(out=rec, in_=sums[h])
                        nc.vector.tensor_copy(out=rec_bf, in_=rec)
                    nc.tensor.matmul(bc[64 * h:64 * (h + 1), :], c1w, rec_bf,
                                     start=True, stop=True)
                attn_w = sb.tile([128, 4, 128], bf16, tag="attnw", bufs=4,
                                 name="attnw")
                with nc.allow_low_precision(reason="bf16 attn"):
                    nc.vector.tensor_tensor(out=attn_w, in0=avp, in1=bc, op=MUL)
                jp = j // 2
                e = 2 * b + jp
                nc.scalar.dma_start(
                    out=send_d[e][4 * (j % 2):4 * (j % 2) + 4].transpose([1, 0, 2]),
                    in_=attn_w)
                if j % 2 == 1:
                    a2a(e)
                    nc.sync.dma_start(out=a2a_sb[e],
                                      in_=recv_d[e][:].transpose([1, 0, 2]))
                    # release proj as PE filler only once the wire has had
                    # ~a full chunk-step to land the exchange
                    ready.setdefault(min(cur_idx + 2, 8), []).extend(
                        proj_ops(b, jp))

            # ---------------- pipelined schedule ----------------
            steps = [(b, t) for b in range(B) for t in range(4)]

            for idx in range(9):
                fill = ready.pop(idx, [])
                if idx < 8:
                    b, tch = steps[idx]
                    if idx == 0:
                        xt = xt0
                    else:
                        col = b * T + 512 * tch
                        xt = sb.tile([128, NK, 512], bf16, tag="xt", bufs=3)
                        nc.sync.dma_start(out=xt[:, 0:4, :],
                                          in_=xT_in[:, 0:4, col:col + 512])
                        nc.scalar.dma_start(out=xt[:, 4:8, :],
                                            in_=xT_in[:, 4:8, col:col + 512])
                    fill = qkv_ops(b, tch, xt) + fill
                fi = [0]

                def pull(n, fill=fill, fi=fi):
                    for _ in range(n):
                        if fi[0] < len(fill):
                            fill[fi[0]]()
                            fi[0] += 1

                if idx == 0:
                    pull(len(fill))
                else:
                    wb, wj = steps[idx - 1]
                    qcol = wb * T + 512 * wj
                    npairs = 2 * (wj + 1)
                    per = max(1, len(fill) // (2 * npairs + 1))
                    av_ps = None
                    for s in range(npairs):
                        P = sb.tile([128, 2, 2, 512], bf16, tag="p", bufs=6,
                                    name="P")
                        for i2 in range(2):
                            m = 2 * s + i2
                            diag = m >= 4 * wj
                            lo = 128 * (m - 4 * wj) if diag else 0
                            sp = ps.tile([128, 2, 512], f32, tag="sp", bufs=2,
                                         name="sp")
                            kcol = wb * T + 128 * m
                            for h in range(HPC):
                                nc.tensor.matmul(
                                    sp[:, h, lo:512],
                                    kT[64 * h:64 * h + D, kcol:kcol + 128],
                                    qT[64 * h:64 * h + D, qcol + lo:qcol + 512],
                                    start=True, stop=True)
                            nc.scalar.activation(out=P[:, i2, :, lo:512],
                                                 in_=sp[:, :, lo:512],
                                                 func=Exp, scale=0.125)
                            if diag:
                                with nc.allow_low_precision(reason="0/1 mask"):
                                    nc.vector.tensor_tensor(
                                        out=P[:, i2, :, lo:lo + 128],
                                        in0=P[:, i2, :, lo:lo + 128],
                                        in1=tri2, op=MUL)
                            pull(per)
                            if s == 0 and i2 == 0:
                                av_ps = [ps.tile([128, 512], f32, tag="av",
                                                 bufs=2, name=f"av{h}")
                                         for h in range(HPC)]
                            for h in range(HPC):
                                nc.tensor.matmul(
                                    av_ps[h][0:D + 1, lo:512],
                                    v_nat[:, 16 * wb + m, VB * h:VB * h + D + 1],
                                    P[:, i2, h, lo:512],
                                    start=(m == 0), stop=(m == 2 * npairs - 1))
                        if s == 0 and pend:
                            do_apply(pend.pop(0), idx)
                    pull(len(fill))
                    avp = sb.tile([128, 512], bf16, tag="avp", bufs=2, name="avp")
                    sums = [sb.tile([1, 512], f32, tag="sums", bufs=4, name="sums")
                            for _ in range(HPC)]
                    with nc.allow_low_precision(reason="bf16 ok"):
                        for h in range(HPC):
                            nc.vector.tensor_copy(out=avp[64 * h:64 * h + D, :],
                                                  in_=av_ps[h][0:D, :])
                            nc.vector.tensor_copy(out=sums[h],
                                                  in_=av_ps[h][D:D + 1, :])
                    pend.append((avp, sums, wb, wj))

            # tail: apply last window, final exchange, remaining proj
            fillt = [op for k in sorted(ready) for op in ready[k]]
            ready.clear()
            half = len(fillt) // 2
            for op in fillt[:half]:
                op()
            do_apply(pend.pop(0), 90)       # (1,3) -> fires a2a(1,1)
            for op in fillt[half:]:
                op()
            for op in [op for k in sorted(ready) for op in ready[k]]:
                op()                        # proj(1,1)
            ready.clear()

    nc.finalize()
    return nc


def kernel(x, Wq, Wk, Wv, Wproj, bproj):
    global LAST_EXEC_NS
    from concourse.bass_utils import run_bass_kernel_spmd

    if "nc" not in _CACHE:
        _CACHE["nc"] = _build()
    nc = _CACHE["nc"]

    bf = ml_dtypes.bfloat16
    xT = np.ascontiguousarray(
        x.reshape(B * T, NK, 128).transpose(2, 1, 0)).astype(bf)
    wp = np.ascontiguousarray(Wproj).astype(bf)

    consts = np.zeros((128, CW_BF), dtype=np.float32)
    consts[:, C_ID:C_ID + 128] = np.eye(128)
    pi = np.arange(128)[:, None]
    ci = np.arange(128)[None, :]
    tri01 = (ci - pi >= 0).astype(np.float32)
    consts[:, C_T2:C_T2 + 128] = tri01
    consts[:, C_T2 + 128:C_T2 + 256] = tri01
    consts[:, C_C1:C_C1 + 64] = 1.0
    consts = consts.astype(bf)

    def rearrange_w(w):
        # [C, CPC] -> [128, NK, CPC]
        return np.ascontiguousarray(
            w.reshape(NK, 128, CPC).transpose(1, 0, 2)).astype(bf)

    in_maps = []
    for c in range(N_CORES):
        in_maps.append({
            "xT": xT,
            "wq": rearrange_w(np.concatenate([Wq[2 * c], Wq[2 * c + 1]], axis=1)),
            "wk": rearrange_w(np.concatenate([Wk[2 * c], Wk[2 * c + 1]], axis=1)),
            "wv": rearrange_w(np.concatenate([Wv[2 * c], Wv[2 * c + 1]], axis=1)),
            "wp": wp,
            "consts": consts,
        })

    res = run_bass_kernel_spmd(nc, in_maps, list(range(N_CORES)))
    LAST_EXEC_NS = res.exec_time_ns
    y = np.empty((B, T, C), dtype=np.float32)
    for c in range(N_CORES):
        yc = np.asarray(res.results[c]["y"]).astype(np.float32) + bproj[None, :]
        for b in range(B):
            for jp in range(2):
                y[b, 1024 * jp + 128 * c:1024 * jp + 128 * c + 128, :] = \
                    yc[128 * (2 * b + jp):128 * (2 * b + jp + 1), :]
    return y
